# revision 6
# baseline (speedup 1.0000x reference)
"""Trainium2 Bass kernel: segment-reduced Euclidean loss.

loss = sum_i ||a_i - b_i||_2 / num_list[seg(i)]   over N rows, D=128.

Strategy (8 NeuronCores, data-parallel):
  - rows are split evenly across the 8 cores (segments stay whole: every
    core boundary is a multiple of the 512-row segments in the graded
    input; for general num_list the per-row weight tensor makes segment
    alignment irrelevant).
  - per core, partition p owns rows [p*q, (p+1)*q) of its shard, so each
    DMA chunk is a [128, u*128] tile whose per-partition source bytes are
    contiguous (u rows x 512B) -- large-burst, full-bandwidth DMA.
  - per chunk: VectorE subtract (in place), ScalarE Square (in place),
    VectorE grouped tensor_reduce over the innermost D=128 -> per-row
    sum-of-squares. DVE ~2 passes and ACT ~1 pass both hide under the
    ~360 GB/s HBM DMA stream.
  - tail: ScalarE Sqrt over the [128, q] sums, multiply by the per-row
    weight 1/num_list[seg(row)] (precomputed on host, DMA'd once),
    row-reduce to [128, 1], DMA out. Host sums 8x128 partials in f64.
"""

import numpy as np

N_ROWS = 1048576
D = 128
N_SEG = 2048
N_CORES = 8
ROWS_PER_CORE = N_ROWS // N_CORES  # 131072
U_DEFAULT = 32  # rows per partition per chunk


def _split_excess_waits(nc, max_waits=1):
    """walrus in this container rejects instructions carrying more than 1
    sync-wait condition ("Too many sync wait commands"). Move excess waits
    onto NoOp carrier instructions inserted just before the offender on the
    same engine -- same-engine program order makes this semantically
    identical."""
    import concourse.mybir as mybir

    for f in nc.m.functions:
        for bb in f.blocks:
            out = []
            changed = False
            for inst in bb.instructions:
                si = inst.sync_info
                waits = list(si.on_wait) if si is not None else []
                if len(waits) > max_waits:
                    keep = waits[-max_waits:]
                    extra = waits[:-max_waits]
                    k = 0
                    while extra:
                        take, extra = extra[:max_waits], extra[max_waits:]
                        nop = mybir.InstNoOp(name=f"{inst.name}-wsplit{k}")
                        nop.engine = inst.engine
                        nop.sync_info = mybir.SyncInfo(on_wait=take, on_update=[])
                        out.append(nop)
                        k += 1
                    inst.sync_info = mybir.SyncInfo(
                        on_wait=keep, on_update=list(si.on_update)
                    )
                    changed = True
                out.append(inst)
            if changed:
                bb.instructions = out


def build_nc(rows_per_core=ROWS_PER_CORE, u=U_DEFAULT, bufs=3):
    """Build the per-core SPMD Bass program (same program on all cores)."""
    import concourse.bass as bass
    import concourse.mybir as mybir
    import concourse.tile as tile

    q = rows_per_core // 128  # rows per partition
    n_chunk = q // u
    assert n_chunk * u == q, (rows_per_core, u)
    f32 = mybir.dt.float32
    AF = mybir.ActivationFunctionType

    nc = bass.Bass("TRN2", target_bir_lowering=False, debug=False)
    a = nc.declare_dram_parameter("a", [rows_per_core, D], f32, isOutput=False)
    b = nc.declare_dram_parameter("b", [rows_per_core, D], f32, isOutput=False)
    w = nc.declare_dram_parameter("w", [128, q], f32, isOutput=False)
    o = nc.declare_dram_parameter("o", [128, 1], f32, isOutput=True)

    av = a.rearrange("(p q) d -> p q d", p=128)
    bv = b.rearrange("(p q) d -> p q d", p=128)

    with tile.TileContext(nc) as tc:
        with (
            tc.tile_pool(name="pa", bufs=bufs) as pa,
            tc.tile_pool(name="pb", bufs=bufs) as pb,
            tc.tile_pool(name="pers", bufs=1) as pp,
        ):
            norms = pp.tile([128, q], f32, tag="norms")
            wt = pp.tile([128, q], f32, tag="wt")
            prod = pp.tile([128, q], f32, tag="prod")
            acc = pp.tile([128, 1], f32, tag="acc")

            nc.sync.dma_start(out=wt[:], in_=w[:])

            for c in range(n_chunk):
                ta = pa.tile([128, u * D], f32)
                tb = pb.tile([128, u * D], f32)
                ta3 = ta[:].rearrange("p (u d) -> p u d", d=D)
                tb3 = tb[:].rearrange("p (u d) -> p u d", d=D)
                nc.sync.dma_start(out=ta3, in_=av[:, c * u : (c + 1) * u, :])
                nc.sync.dma_start(out=tb3, in_=bv[:, c * u : (c + 1) * u, :])
                nc.vector.tensor_sub(ta[:], ta[:], tb[:])
                nc.scalar.activation(ta[:], ta[:], AF.Square)
                nc.vector.tensor_reduce(
                    norms[:, c * u : (c + 1) * u],
                    ta3,
                    axis=mybir.AxisListType.X,
                    op=mybir.AluOpType.add,
                )

            nc.scalar.activation(norms[:], norms[:], AF.Sqrt)
            nc.vector.tensor_mul(prod[:], norms[:], wt[:])
            nc.vector.tensor_reduce(
                acc[:], prod[:], axis=mybir.AxisListType.X, op=mybir.AluOpType.add
            )
            nc.sync.dma_start(out=o[:], in_=acc[:])

    _split_excess_waits(nc)
    return nc


def build_empty_nc():
    """Minimal program (memset + 4KB DMA out) for launch-overhead calibration."""
    import concourse.bass as bass
    import concourse.mybir as mybir
    import concourse.tile as tile

    f32 = mybir.dt.float32
    nc = bass.Bass("TRN2", target_bir_lowering=False, debug=False)
    o = nc.declare_dram_parameter("o", [128, 1], f32, isOutput=True)
    with tile.TileContext(nc) as tc:
        with tc.tile_pool(name="p", bufs=1) as pp:
            acc = pp.tile([128, 1], f32)
            nc.vector.memset(acc[:], 0.0)
            nc.sync.dma_start(out=o[:], in_=acc[:])
    _split_excess_waits(nc)
    return nc


_CACHE = {}


def _get_nc(rows_per_core, u):
    key = (rows_per_core, u)
    if key not in _CACHE:
        _CACHE[key] = build_nc(rows_per_core, u)
    return _CACHE[key]


def _seg_ids(num_list, n_rows):
    """np.repeat with jnp.repeat(total_repeat_length=n) pad/truncate semantics."""
    nl = np.asarray(num_list, dtype=np.int64)
    full = np.repeat(np.arange(nl.shape[0], dtype=np.int64), nl)
    if full.size >= n_rows:
        return full[:n_rows]
    pad_val = full[-1] if full.size else 0
    return np.concatenate([full, np.full(n_rows - full.size, pad_val, np.int64)])


def make_in_maps(clip_remap, clip_emb, num_list, rows_per_core=ROWS_PER_CORE):
    a = np.ascontiguousarray(np.asarray(clip_remap, dtype=np.float32))
    b = np.ascontiguousarray(np.asarray(clip_emb, dtype=np.float32))
    n_rows = a.shape[0]
    nl = np.asarray(num_list)
    seg = _seg_ids(nl, n_rows)
    denom = nl[seg].astype(np.float32)
    wrow = (np.float32(1.0) / denom).astype(np.float32)
    q = rows_per_core // 128
    in_maps = []
    for c in range(N_CORES):
        lo, hi = c * rows_per_core, (c + 1) * rows_per_core
        in_maps.append(
            {
                "a": a[lo:hi],
                "b": b[lo:hi],
                "w": np.ascontiguousarray(wrow[lo:hi].reshape(128, q)),
            }
        )
    return in_maps


def kernel(clip_remap, clip_emb, num_list):
    from concourse.bass_utils import run_bass_kernel_spmd

    a = np.asarray(clip_remap)
    rows_per_core = a.shape[0] // N_CORES
    nc = _get_nc(rows_per_core, U_DEFAULT)
    in_maps = make_in_maps(clip_remap, clip_emb, num_list, rows_per_core)
    res = run_bass_kernel_spmd(nc, in_maps, core_ids=list(range(N_CORES)))
    total = np.float64(0.0)
    for r in res.results:
        total += r["o"].astype(np.float64).sum()
    return np.asarray(total, dtype=np.float32)
